# revision 1
# baseline (speedup 1.0000x reference)
"""Trainium2 Bass kernel for nn_BHS_SAGE (GNN message passing + dueling head).

Data-parallel over the batch of 128 graphs: 16 graphs per NeuronCore x 8 cores.
All weights replicated. Inputs are host-marshalled (transposed / packed) into
per-core layouts; device does all the math.

Pipeline per core (16 graphs, N=1024 nodes, F=32, H=128, DEG=16):
  A. hp = relu(x @ W_pool.T + b_pool)  -> bf16, one DMA to DRAM [node,(g,f)]
  B. agg = max over 16 in-edges: 16 indirect-DMA gathers (bypass) folded with
     DVE max in two chains (no CCE compute on indirect DMA on trn2)
  C. PE transposes agg -> aggT [f, node] (bf16)
  D. h = relu(x@W_self.T + agg@W_neigh.T + b_sage) -> H_T [128h, (g,n)] fp32
  E. head: psum[16g, 76] += H_T[:, n-slice].T @ W_chunk (1024 K-steps,
     W = [W_adv; W_v1] streamed from DRAM in [o, n, a] layout)
  F. tail: dueling combine (adv mean, val MLP 64->64->1) -> out [16, 12]
"""

import numpy as np

B, N, F, H, DEG = 128, 1024, 32, 128, 16
NCORES = 8
BL = B // NCORES          # 16 graphs per core
NA = 12                   # adv outputs (3 branches x 4 actions)
NV = 64                   # val hidden
NH = NA + NV              # 76 combined head outputs
GROUPS = BL // 4          # 4 graphs packed per 128 partitions
GF = BL * F               # 512: free size of one node's (g,f) row

_CACHE = {}
LAST_RESULTS = None


def _build_program():
    import concourse.bass as bass
    import concourse.bacc as bacc
    import concourse.mybir as mybir
    import concourse.tile as tile

    f32 = mybir.dt.float32
    bf16 = mybir.dt.bfloat16
    i32 = mybir.dt.int32
    Relu = mybir.ActivationFunctionType.Relu
    Alu = mybir.AluOpType

    nc = bacc.Bacc("TRN2", target_bir_lowering=False, debug=False,
                   num_devices=NCORES)

    # ---- kernel I/O ----
    xt_d = nc.declare_dram_parameter("xt", [128, GROUPS * N], bf16, isOutput=False)
    xe_d = nc.declare_dram_parameter("xe", [128, DEG * GROUPS * N], bf16, isOutput=False)
    wpool_d = nc.declare_dram_parameter("wpool_bd", [128, 128], bf16, isOutput=False)
    bpool_d = nc.declare_dram_parameter("bpool", [128, 1], f32, isOutput=False)
    wself_d = nc.declare_dram_parameter("wself_bd", [128, 4 * H], bf16, isOutput=False)
    wneigh_d = nc.declare_dram_parameter("wneigh_bd", [128, 4 * H], bf16, isOutput=False)
    bsage_d = nc.declare_dram_parameter("bsage", [128, 1], f32, isOutput=False)
    identf_d = nc.declare_dram_parameter("identf", [128, 128], f32, isOutput=False)
    whead_d = nc.declare_dram_parameter("whead", [128, N * NH], bf16, isOutput=False)
    badv_d = nc.declare_dram_parameter("badv", [BL, NA], f32, isOutput=False)
    bv1_d = nc.declare_dram_parameter("bv1", [BL, NV], f32, isOutput=False)
    wv2_d = nc.declare_dram_parameter("wv2", [NV, NV], f32, isOutput=False)
    bv2_d = nc.declare_dram_parameter("bv2", [NV, 1], f32, isOutput=False)
    wv3_d = nc.declare_dram_parameter("wv3", [NV, 1], f32, isOutput=False)
    bv3_d = nc.declare_dram_parameter("bv3r", [BL, 1], f32, isOutput=False)
    out_d = nc.declare_dram_parameter("out", [BL, NA], f32, isOutput=True)

    WCH = 128                      # head-weight chunk: nodes per streamed tile
    NCHUNK = N // WCH              # 8 chunks

    import os as _os
    _dbg = _os.environ.get("KDBG") == "1"
    if _dbg:
        dbg_aggT_d = nc.declare_dram_parameter("dbg_aggT", [128, GROUPS * N], bf16, isOutput=True)
        dbg_ht_d = nc.declare_dram_parameter("dbg_ht", [128, BL * N], bf16, isOutput=True)

    with tile.TileContext(nc) as tc:
        with (
            tc.tile_pool(name="const", bufs=1) as cpool,
            tc.tile_pool(name="big", bufs=1) as bigpool,
        ):
            # ---- constants / persistent tiles ----
            identf = cpool.tile([128, 128], f32)
            nc.sync.dma_start(out=identf[:], in_=identf_d[:])
            xt = cpool.tile([128, GROUPS * N], bf16)
            nc.sync.dma_start(out=xt[:], in_=xt_d[:])
            wpool = cpool.tile([128, 128], bf16)
            nc.sync.dma_start(out=wpool[:], in_=wpool_d[:])
            bpool = cpool.tile([128, 1], f32)
            nc.sync.dma_start(out=bpool[:], in_=bpool_d[:])
            wself = cpool.tile([128, 4 * H], bf16)
            nc.sync.dma_start(out=wself[:], in_=wself_d[:])
            wneigh = cpool.tile([128, 4 * H], bf16)
            nc.sync.dma_start(out=wneigh[:], in_=wneigh_d[:])
            bsage = cpool.tile([128, 1], f32)
            nc.sync.dma_start(out=bsage[:], in_=bsage_d[:])

            ht = bigpool.tile([128, BL * N], bf16)       # H_T: [h, g*1024+n] 4MB
            aggT = bigpool.tile([128, GROUPS * N], bf16)  # [(q,o), grp*1024+n] 1MB

            # head psum allocated up-front so the head stage can overlap the
            # h stage (no PSUM space-reuse dependency between their pools)
            hd_ps_ctx = tc.tile_pool(name="hd_ps", bufs=1, space="PSUM")
            hd_ps = hd_ps_ctx.__enter__()
            pshead = hd_ps.tile([BL, NH], f32)

            # ---- stage A+B fused: aggT = relu(max_d(W_pool @ x[src_d]) + b) ----
            # relu(.+b) is monotone, so the max moves inside. W_pool is the
            # stationary operand, so z comes out feature-major (= aggT layout,
            # no transposes) and bias is per-partition (fused in one ACT op).
            # xe is host-gathered edge-ordered x, 16 edge slots interleaved
            # per node so one DVE reduce_max folds a whole 32-node block.
            SLAB = 512 * DEG                         # 512 nodes x 16 d cols
            with (
                tc.tile_pool(name="xe_sb", bufs=3) as xe_pool,
                tc.tile_pool(name="z_ps", bufs=4, space="PSUM") as z_ps,
            ):
                for s in range(2 * GROUPS):          # slab = (grp, half)
                    grp, half = s // 2, s % 2
                    xe = xe_pool.tile([128, SLAB], bf16, tag="xe")
                    nc.sync.dma_start(
                        out=xe[:], in_=xe_d[:, s * SLAB:(s + 1) * SLAB])
                    for blk in range(16):            # 32 nodes x 16 d per blk
                        ps = z_ps.tile([128, 512], f32, tag="zps")
                        nc.tensor.matmul(
                            out=ps[:],
                            lhsT=wpool[:],
                            rhs=xe[:, blk * 512:(blk + 1) * 512],
                            start=True, stop=True,
                        )
                        nc.vector.reduce_max(
                            out=aggT[:, grp * N + half * 512 + blk * 32:
                                     grp * N + half * 512 + (blk + 1) * 32],
                            in_=ps[:].rearrange("p (n d) -> p n d", d=DEG),
                            axis=mybir.AxisListType.X)
            # fused bias + relu (per-partition bias)
            for grp in range(GROUPS):
                nc.scalar.activation(
                    out=aggT[:, grp * N:(grp + 1) * N],
                    in_=aggT[:, grp * N:(grp + 1) * N],
                    func=Relu, bias=bpool[:])

            if _dbg:
                nc.sync.dma_start(out=dbg_aggT_d[:], in_=aggT[:])

            # ---- stage D: H_T = relu(W_self x + W_neigh agg + b_sage) ----
            with tc.tile_pool(name="h_ps", bufs=2, space="PSUM") as h_ps:
                for half in range(2):
                    for g in range(BL):
                        q, grp = g % 4, g // 4
                        base = grp * N + half * 512
                        ps = h_ps.tile([128, 512], f32, tag="hps")
                        # zero-padded K=128 weights select graph g's quadrant
                        nc.tensor.matmul(
                            out=ps[:],
                            lhsT=wself[:, q * H:(q + 1) * H],
                            rhs=xt[:, base: base + 512],
                            start=True, stop=False)
                        nc.tensor.matmul(
                            out=ps[:],
                            lhsT=wneigh[:, q * H:(q + 1) * H],
                            rhs=aggT[:, base: base + 512],
                            start=False, stop=True)
                        nc.scalar.activation(
                            out=ht[:, g * N + half * 512: g * N + half * 512 + 512],
                            in_=ps[:], func=Relu, bias=bsage[:])

            if _dbg:
                nc.sync.dma_start(out=dbg_ht_d[:], in_=ht[:])

            # ---- stage E: head psum[16, 76] += H_T-slice.T @ W-chunk ----
            htv = ht[:].rearrange("p (g n) -> p n g", n=N)     # [128, 1024, 16]
            with (
                tc.tile_pool(name="wst", bufs=3) as wpool_st,
            ):
                for c in range(NCHUNK):
                    wt = wpool_st.tile([128, WCH * NH], bf16, tag="wt")
                    nc.sync.dma_start(
                        out=wt[:], in_=whead_d[:, c * WCH * NH:(c + 1) * WCH * NH])
                    for j in range(WCH):
                        n = c * WCH + j
                        nc.tensor.matmul(
                            out=pshead[:],
                            lhsT=htv[:, n: n + 1, :],
                            rhs=wt[:, j * NH:(j + 1) * NH],
                            start=(n == 0), stop=(n == N - 1),
                        )

                # ---- stage F: dueling tail ----
                with tc.tile_pool(name="tail", bufs=1) as tp:
                    badv = tp.tile([BL, NA], f32)
                    nc.sync.dma_start(out=badv[:], in_=badv_d[:])
                    bv1 = tp.tile([BL, NV], f32)
                    nc.sync.dma_start(out=bv1[:], in_=bv1_d[:])
                    wv2 = tp.tile([NV, NV], f32)
                    nc.sync.dma_start(out=wv2[:], in_=wv2_d[:])
                    bv2 = tp.tile([NV, 1], f32)
                    nc.sync.dma_start(out=bv2[:], in_=bv2_d[:])
                    wv3 = tp.tile([NV, 1], f32)
                    nc.sync.dma_start(out=wv3[:], in_=wv3_d[:])
                    bv3 = tp.tile([BL, 1], f32)
                    nc.sync.dma_start(out=bv3[:], in_=bv3_d[:])

                    adv = tp.tile([BL, NA], f32)
                    nc.vector.tensor_tensor(
                        out=adv[:], in0=pshead[:, 0:NA], in1=badv[:], op=Alu.add)
                    nc.vector.tensor_scalar_max(adv[:], adv[:], 0.0)
                    val1 = tp.tile([BL, NV], f32)
                    nc.vector.tensor_tensor(
                        out=val1[:], in0=pshead[:, NA:NH], in1=bv1[:], op=Alu.add)
                    nc.vector.tensor_scalar_max(val1[:], val1[:], 0.0)

                    with tc.tile_pool(name="tl_ps", bufs=2, space="PSUM") as tl_ps:
                        # val1 [16, 64] -> val1T [64, 16]
                        pst = tl_ps.tile([NV, BL], f32, tag="a")
                        nc.tensor.transpose(
                            out=pst[:], in_=val1[:], identity=identf[0:BL, 0:BL])
                        val1T = tp.tile([NV, BL], f32)
                        nc.scalar.copy(out=val1T[:], in_=pst[:])
                        # val2T [64, 16] = relu(W_v2 @ val1 + b_v2)
                        ps2 = tl_ps.tile([NV, BL], f32, tag="b")
                        nc.tensor.matmul(
                            out=ps2[:], lhsT=wv2[:], rhs=val1T[:], start=True, stop=True)
                        val2T = tp.tile([NV, BL], f32)
                        nc.scalar.activation(
                            out=val2T[:], in_=ps2[:], func=Relu, bias=bv2[:])
                        # val3 [16, 1]
                        ps3 = tl_ps.tile([BL, 1], f32, tag="a")
                        nc.tensor.matmul(
                            out=ps3[:], lhsT=val2T[:], rhs=wv3[:], start=True, stop=True)
                        val3 = tp.tile([BL, 1], f32)
                        nc.vector.tensor_tensor(
                            out=val3[:], in0=ps3[:], in1=bv3[:], op=Alu.add)

                    # out = val + adv - mean_j(adv)
                    m = tp.tile([BL, 3], f32)
                    nc.vector.reduce_sum(
                        out=m[:],
                        in_=adv[:].rearrange("p (a b) -> p a b", b=4),
                        axis=mybir.AxisListType.X)
                    nc.vector.tensor_scalar_mul(m[:], m[:], 0.25)
                    outt = tp.tile([BL, NA], f32)
                    nc.vector.tensor_tensor(
                        out=outt[:], in0=adv[:],
                        in1=val3[:].to_broadcast([BL, NA]), op=Alu.add)
                    nc.vector.tensor_tensor(
                        out=outt[:].rearrange("p (a b) -> p a b", b=4),
                        in0=outt[:].rearrange("p (a b) -> p a b", b=4),
                        in1=m[:].to_broadcast([BL, 3, 4]),
                        op=Alu.subtract)
                    nc.sync.dma_start(out=out_d[:], in_=outt[:])
            hd_ps_ctx.__exit__(None, None, None)
    nc.compile()
    return nc


def _make_in_maps(inputs):
    import ml_dtypes
    bf = ml_dtypes.bfloat16

    x = np.asarray(inputs["x"], np.float32)
    src = np.asarray(inputs["src"], np.int32)
    W_pool = np.asarray(inputs["W_pool"], np.float32)
    b_pool = np.asarray(inputs["b_pool"], np.float32)
    W_self = np.asarray(inputs["W_self"], np.float32)
    W_neigh = np.asarray(inputs["W_neigh"], np.float32)
    b_sage = np.asarray(inputs["b_sage"], np.float32)
    W_adv = np.asarray(inputs["W_adv"], np.float32)
    b_adv = np.asarray(inputs["b_adv"], np.float32)
    W_v1 = np.asarray(inputs["W_v1"], np.float32)
    b_v1 = np.asarray(inputs["b_v1"], np.float32)
    W_v2 = np.asarray(inputs["W_v2"], np.float32)
    b_v2 = np.asarray(inputs["b_v2"], np.float32)
    W_v3 = np.asarray(inputs["W_v3"], np.float32)
    b_v3 = np.asarray(inputs["b_v3"], np.float32)

    # shared (replicated) tensors
    wpool_bd = np.kron(np.eye(4, dtype=np.float32), W_pool.T)                # [128, 128]
    wpool_bd = np.ascontiguousarray(wpool_bd).astype(bf)
    bpool = np.ascontiguousarray(np.tile(b_pool, 4)[:, None], np.float32)    # [128, 1]
    wself_bd = np.zeros((128, 4 * H), np.float32)                            # [128, 512]
    wneigh_bd = np.zeros((128, 4 * H), np.float32)
    for q in range(4):
        wself_bd[q * 32:(q + 1) * 32, q * H:(q + 1) * H] = W_self.T
        wneigh_bd[q * 32:(q + 1) * 32, q * H:(q + 1) * H] = W_neigh.T
    bsage = np.ascontiguousarray(b_sage[:, None])                            # [128, 1]
    W_cat = np.concatenate([W_adv, W_v1], axis=0)                            # [76, 131072]
    whead = np.ascontiguousarray(
        W_cat.reshape(NH, N, H).transpose(2, 1, 0).reshape(H, N * NH)).astype(bf)
    badv = np.ascontiguousarray(np.broadcast_to(b_adv[None, :], (BL, NA)))
    bv1 = np.ascontiguousarray(np.broadcast_to(b_v1[None, :], (BL, NV)))
    wv2 = np.ascontiguousarray(W_v2.T)                                       # [64, 64]
    bv2 = np.ascontiguousarray(b_v2[:, None])                                # [64, 1]
    wv3 = np.ascontiguousarray(W_v3.T)                                       # [64, 1]
    bv3r = np.full((BL, 1), float(b_v3[0]), np.float32)
    ident = np.eye(128, dtype=np.float32)

    shared = {
        "wpool_bd": wpool_bd, "bpool": bpool,
        "wself_bd": wself_bd.astype(bf), "wneigh_bd": wneigh_bd.astype(bf),
        "bsage": bsage, "whead": whead, "badv": badv,
        "bv1": bv1, "wv2": wv2, "bv2": bv2, "wv3": wv3, "bv3r": bv3r,
        "identf": ident,
    }

    in_maps = []
    for c in range(NCORES):
        xs = x[c * BL:(c + 1) * BL]                                          # [16,1024,32]
        xt = np.ascontiguousarray(
            xs.reshape(GROUPS, 4, N, F).transpose(1, 3, 0, 2)
            .reshape(128, GROUPS * N)).astype(bf)
        g0 = c * BL
        sb = src[g0 * N * DEG:(g0 + 1) * N * DEG] - g0 * N                   # local [16384]
        idx = sb.reshape(N, DEG)                           # idx[n, d]
        # host edge-expansion of x: slab s = (grp, half) holds 512 nodes
        # with the 16 edge slots interleaved per node: col = (n-n0)*16 + d
        xe = np.empty((128, DEG * GROUPS * N), bf)
        SLAB = 512 * DEG
        for s in range(2 * GROUPS):
            grp, half = s // 2, s % 2
            sub = idx[half * 512:(half + 1) * 512, :]      # [512, 16]
            xg = xs[grp * 4:(grp + 1) * 4, sub, :]         # [4q, 512n, 16d, 32f]
            xe[:, s * SLAB:(s + 1) * SLAB] = (
                xg.transpose(0, 3, 1, 2)                   # [q, f, n, d]
                .reshape(128, SLAB).astype(bf))
        in_maps.append({"xt": xt, "xe": xe, **shared})
    return in_maps


def kernel(**inputs) -> np.ndarray:
    global LAST_RESULTS
    from concourse.bass_utils import run_bass_kernel_spmd

    if "nc" not in _CACHE:
        _CACHE["nc"] = _build_program()
    nc = _CACHE["nc"]
    in_maps = _make_in_maps(inputs)
    rr = run_bass_kernel_spmd(nc, in_maps, list(range(NCORES)))
    LAST_RESULTS = rr
    out = np.zeros((B, 3, 4), np.float32)
    for c in range(NCORES):
        out[c * BL:(c + 1) * BL] = rr.results[c]["out"].reshape(BL, 3, 4)
    return out

